# revision 13
# baseline (speedup 1.0000x reference)
"""Trainium2 Bass kernel for nn_AverageAttn (B=4, S=4096, D=H=1024, 8 cores).

out = igate * iQ + fgate * h, where
  avg  = causal cumulative average of iV along seq
  h    = relu(avg @ W1 + b1) @ W2 + b2
  ifg  = sigmoid(concat(iQ, h) @ Wg + bg);  igate, fgate = split(ifg)

Sharding: 8 cores = (batch b, seq half hf).  Each core processes 2048 tokens.
Cores with hf=1 also stream iV[b, :2048] to build the prefix chunk-sums.

Speed levers vs the f32r baseline:
  * FFN1/FFN2/gate matmuls run in fp8(e4m3) DoubleRow mode (2 rows/cycle);
    activations are scaled by 16 and weights by 64 before quantization, and
    the scalar-engine activations descale via their `scale` argument.
  * iQ arrives host-pretransposed ([feature, token] tiles) in both f32 and
    e4m3, and the output leaves in T-orientation (host un-transposes) — no
    PE transposes at all.
  * All weights live in SBUF for the whole kernel (fp8 slabs, ~48KB/part).
  * The whole cumsum path (v, ltri/mask tables, S table, oband) is bf16 so
    the N=128 cumsum matmuls run at 1 cycle/row (f32r moving operands at
    N<256 run 4x slower, and walrus rejects mixed f32r/bf16 matmuls).
"""

import numpy as np

B, S, D = 4, 4096, 1024
H = 1024
T = S // 2              # tokens per core
P = 128
NCH = T // P            # 16 chunks of 128 tokens per core
NBLK = 4                # 512-token blocks per core
CPB = 4                 # chunks per block
ND = D // P             # 8 feature chunks
NG = 2 * D // P         # 16 gate chunks
SCA = 16.0              # fp8 activation scale
SCW = 64.0              # fp8 weight scale

GATE_FP8 = True         # gate iQ-half in fp8 DoubleRow (vs bf16 1x)


def _host_constants():
    """Per-parity constants: scaled triangular blocks and carry masks."""
    import ml_dtypes
    bf = ml_dtypes.bfloat16
    consts = {}
    for half in (0, 1):
        off = half * T
        # ltri[t, c, s] = 1/(off + 128c + s + 1) if t <= s else 0
        ltri = np.zeros((P, NCH, P), np.float32)
        t = np.arange(P)[:, None]
        s = np.arange(P)[None, :]
        for c in range(NCH):
            denom = 1.0 / (off + P * c + s + 1).astype(np.float32)
            ltri[:, c, :] = np.where(t <= s, denom, 0.0)
        # mask[r, b, s] = 1/(off + 512b + s + 1) if S-row r feeds chunk of s
        mask = np.zeros((P, NBLK, 4 * P), np.float32)
        sb = np.arange(4 * P)
        for b in range(NBLK):
            w = 1.0 / (off + 4 * P * b + sb + 1).astype(np.float32)
            cc = sb // P  # chunk-in-block of each s
            for r in range(32):
                if r < 16:
                    inc = np.full(4 * P, half == 1)
                else:
                    inc = (r - 16) < (4 * b + cc)
                mask[r, b, :] = np.where(inc, w, 0.0)
        ltri_b = np.ascontiguousarray(
            ltri.reshape(P, NBLK, CPB, P).transpose(1, 0, 2, 3)).astype(bf)
        mask_b = np.ascontiguousarray(mask.transpose(1, 0, 2)).astype(bf)
        consts[half] = (ltri_b, mask_b)
    # oband[p, i] = 1 iff i == 32  ->  lhsT for S-row r is oband[:, 32-r:160-r]
    oband = np.zeros((P, 160), np.float32)
    oband[:, 32] = 1.0
    return consts, oband.astype(bf)


def _build_program(gate_fp8=True):
    import concourse.bass as bass  # noqa: F401
    import contextlib
    import concourse.tile as tile
    from concourse import mybir, bacc

    f32 = mybir.dt.float32
    f32r = mybir.dt.float32r
    bf16 = mybir.dt.bfloat16
    f8 = mybir.dt.float8e4
    DR = mybir.MatmulPerfMode.DoubleRow
    Relu = mybir.ActivationFunctionType.Relu
    Ident = mybir.ActivationFunctionType.Identity
    Sigm = mybir.ActivationFunctionType.Sigmoid

    nc = bacc.Bacc("TRN2", target_bir_lowering=False)

    v = nc.dram_tensor("v", [NCH, P, D], bf16, kind="ExternalInput")
    vpre = nc.dram_tensor("vpre", [NCH, P, D], bf16, kind="ExternalInput")
    iqt = nc.dram_tensor("iqt", [NBLK, ND, P, 4 * P], f32, kind="ExternalInput")
    iqt8 = nc.dram_tensor("iqt8", [NBLK, ND, P, 4 * P], f8, kind="ExternalInput")
    w1s = nc.dram_tensor("w1s", [ND, P, ND, P], f8, kind="ExternalInput")
    w2s = nc.dram_tensor("w2s", [ND, P, ND, P], f8, kind="ExternalInput")
    wgs = nc.dram_tensor("wgs", [NG, P, NG, P], f8, kind="ExternalInput")
    if not gate_fp8:
        # bf16 slabs of 1024*Wg[:D] so the psum stays at 1024*z
        wgb = nc.dram_tensor("wgb", [NG, P, ND, P], bf16, kind="ExternalInput")
    b1c = nc.dram_tensor("b1c", [P, ND], f32, kind="ExternalInput")   # SCA*b1
    b2cr = nc.dram_tensor("b2cr", [P, ND], f32, kind="ExternalInput")  # b2
    b2cs = nc.dram_tensor("b2cs", [P, ND], f32, kind="ExternalInput")  # SCA*b2
    bgc = nc.dram_tensor("bgc", [P, NG], f32, kind="ExternalInput")
    ltri = nc.dram_tensor("ltri", [NBLK, P, CPB, P], bf16, kind="ExternalInput")
    maskd = nc.dram_tensor("maskd", [NBLK, P, 4 * P], bf16, kind="ExternalInput")
    oband = nc.dram_tensor("oband", [P, 160], bf16, kind="ExternalInput")
    o = nc.dram_tensor("o", [NBLK, ND, P, 4 * P], f32, kind="ExternalOutput")

    with tile.TileContext(nc) as tc:
        ctx = contextlib.ExitStack()
        with ctx:
            cpool = ctx.enter_context(tc.tile_pool(name="consts", bufs=1))
            vpool = ctx.enter_context(tc.tile_pool(name="vq", bufs=6))
            qpool = ctx.enter_context(tc.tile_pool(name="qp", bufs=2))
            q8pool = ctx.enter_context(tc.tile_pool(name="q8", bufs=2))
            apool = ctx.enter_context(tc.tile_pool(name="avgq", bufs=2))
            h1pool = ctx.enter_context(tc.tile_pool(name="h1q", bufs=2))
            hqpool = ctx.enter_context(tc.tile_pool(name="hqp", bufs=2))
            hfpool = ctx.enter_context(tc.tile_pool(name="hfp", bufs=2))
            gpool = ctx.enter_context(tc.tile_pool(name="gates", bufs=3))
            opool = ctx.enter_context(tc.tile_pool(name="outs", bufs=3))
            ps_sp = ctx.enter_context(tc.tile_pool(name="pssp", bufs=1, space="PSUM"))
            ps_cum = ctx.enter_context(tc.tile_pool(name="pscum", bufs=2, space="PSUM"))
            ps_mm = ctx.enter_context(tc.tile_pool(name="psmm", bufs=4, space="PSUM"))

            # ---- startup-critical tables, split across both HWDGE queues --
            obandT = cpool.tile([P, 160], bf16, tag="oband")
            nc.sync.dma_start(obandT[:], oband[:])
            ltriT = cpool.tile([P, NBLK, CPB, P], bf16, tag="ltri")
            nc.scalar.dma_start(ltriT[:], ltri[:].rearrange("b p c q -> p b c q"))
            maskT = cpool.tile([P, NBLK, 4 * P], bf16, tag="mask")
            nc.scalar.dma_start(maskT[:], maskd[:].rearrange("b p s -> p b s"))
            b1T = cpool.tile([P, ND], f32, tag="b1")
            nc.scalar.dma_start(b1T[:], b1c[:])
            b2rT = cpool.tile([P, ND], f32, tag="b2r")
            nc.scalar.dma_start(b2rT[:], b2cr[:])
            b2sT = cpool.tile([P, ND], f32, tag="b2s")
            nc.scalar.dma_start(b2sT[:], b2cs[:])
            bgT = cpool.tile([P, NG], f32, tag="bg")
            nc.scalar.dma_start(bgT[:], bgc[:])

            S_sb = cpool.tile([P, D], bf16, tag="Ssb")

            iqT = {}
            iq8T = {}
            avgq = {}
            h1q = {}
            hqT = {}
            hfT = {}
            ig_sb = {}

            # ---- prefix pass: S rows 0..15 from vpre ----------------------
            # vpre chunks alternate between the two HWDGE queues so the
            # stream isn't serialized behind one queue.
            sp = ps_sp.tile([P, D], f32, tag="sp")
            for c in range(NCH):
                vch = vpool.tile([P, D], bf16, tag="vch")
                eng = nc.sync if c % 2 == 0 else nc.scalar
                eng.dma_start(vch[:], vpre[c])
                for hf in range(2):
                    nc.tensor.matmul(
                        sp[:, hf * 512:(hf + 1) * 512],
                        obandT[:, 32 - c:160 - c],
                        vch[:, hf * 512:(hf + 1) * 512],
                        start=(c == 0), stop=(c == NCH - 1),
                        skip_group_check=True,
                    )
            nc.vector.tensor_copy(S_sb[:], sp[:])

            # ---- resident weights (needed from ffn_block(0) onwards) ------
            w1T = cpool.tile([P, ND, ND, P], f8, tag="w1")
            nc.scalar.dma_start(w1T[:], w1s[:].rearrange("j p k q -> p j k q"))
            w2T = cpool.tile([P, ND, ND, P], f8, tag="w2")
            nc.scalar.dma_start(w2T[:], w2s[:].rearrange("j p k q -> p j k q"))
            wgT = cpool.tile([P, NG, NG, P], f8, tag="wg")
            nc.scalar.dma_start(wgT[:], wgs[:].rearrange("j p k q -> p j k q"))
            if not gate_fp8:
                wgbT = cpool.tile([P, NG, ND, P], bf16, tag="wgb")
                nc.scalar.dma_start(wgbT[:], wgb[:].rearrange("j p k q -> p j k q"))

            def scan_block(blk):
                """Stream v + iqt, S-rows, cumulative average -> avgq fp8."""
                vchs = []
                for cc in range(CPB):
                    c = blk * CPB + cc
                    vch = vpool.tile([P, D], bf16, tag="vch")
                    nc.sync.dma_start(vch[:], v[c])
                    vchs.append(vch)
                iq = qpool.tile([P, ND, 4 * P], f32r, tag="iqt")
                nc.sync.dma_start(
                    iq[:], iqt[blk].bitcast(f32r).rearrange("n p t -> p n t"))
                iqT[blk] = iq
                iq8 = q8pool.tile([P, ND, 4 * P], f8, tag="iqt8")
                nc.sync.dma_start(iq8[:], iqt8[blk].rearrange("n p t -> p n t"))
                iq8T[blk] = iq8

                sp = ps_sp.tile([P, D], f32, tag="sp")
                for cc in range(CPB):
                    r = 16 + blk * CPB + cc
                    for hf in range(2):
                        nc.tensor.matmul(
                            sp[:, hf * 512:(hf + 1) * 512],
                            obandT[:, 32 - r:160 - r],
                            vchs[cc][:, hf * 512:(hf + 1) * 512],
                            start=(cc == 0), stop=(cc == CPB - 1),
                            skip_group_check=True,
                        )
                nc.vector.tensor_add(S_sb[:], S_sb[:], sp[:])

                aq = apool.tile([P, ND, 4 * P], f8, tag="avgq")
                for d in range(ND):
                    pav = ps_cum.tile([P, 4 * P], f32, tag="avg")
                    # cc=0 clears the whole bank (start=True); cc=1..3 land on
                    # has_written=0 slices (overwrite); carry accumulates last.
                    for cc in range(CPB):
                        nc.tensor.matmul(
                            pav[:, cc * P:(cc + 1) * P],
                            vchs[cc][:, d * P:(d + 1) * P],
                            ltriT[:, blk, cc, :],
                            start=(cc == 0), stop=False,
                            skip_group_check=True,
                        )
                    nc.tensor.matmul(
                        pav[:],
                        S_sb[:, d * P:(d + 1) * P],
                        maskT[:, blk, :],
                        start=False, stop=True,
                        skip_group_check=True,
                    )
                    nc.scalar.activation(aq[:, d, :], pav[:], Ident, scale=SCA)
                avgq[blk] = aq

            def ffn_block(blk):
                h1 = h1pool.tile([P, ND, 4 * P], f8, tag="h1q")
                for j in range(ND):
                    pm = ps_mm.tile([P, 4 * P], f32, tag="mm")
                    for dp in range(ND // 2):
                        nc.tensor.matmul(
                            pm[:], w1T[:, j, 2 * dp:2 * dp + 2, :],
                            avgq[blk][:, 2 * dp:2 * dp + 2, :],
                            start=(dp == 0), stop=(dp == ND // 2 - 1),
                            perf_mode=DR,
                        )
                    nc.scalar.activation(h1[:, j, :], pm[:], Relu,
                                         bias=b1T[:, j:j + 1], scale=1.0 / SCW)
                h1q[blk] = h1

                hq = hqpool.tile([P, ND, 4 * P], f8, tag="hq")
                hf = hfpool.tile([P, ND, 4 * P], bf16, tag="hf")
                for d2 in range(ND):
                    pm = ps_mm.tile([P, 4 * P], f32, tag="mm")
                    for jp in range(ND // 2):
                        nc.tensor.matmul(
                            pm[:], w2T[:, d2, 2 * jp:2 * jp + 2, :],
                            h1[:, 2 * jp:2 * jp + 2, :],
                            start=(jp == 0), stop=(jp == ND // 2 - 1),
                            perf_mode=DR,
                        )
                    nc.scalar.activation(hf[:, d2, :], pm[:], Ident,
                                         bias=b2rT[:, d2:d2 + 1],
                                         scale=1.0 / (SCA * SCW))
                    nc.scalar.activation(hq[:, d2, :], pm[:], Ident,
                                         bias=b2sT[:, d2:d2 + 1],
                                         scale=1.0 / SCW)
                hqT[blk] = hq
                hfT[blk] = hf

            def gate_block(blk):
                for gp in range(ND):
                    for gg in (gp, gp + ND):
                        pg = ps_mm.tile([P, 4 * P], f32, tag="mm")
                        if gate_fp8:
                            for kp in range(NG // 2):
                                rhs = (iq8T[blk][:, 2 * kp:2 * kp + 2, :]
                                       if kp < ND // 2 else
                                       hqT[blk][:, 2 * kp - ND:2 * kp - ND + 2, :])
                                nc.tensor.matmul(
                                    pg[:], wgT[:, gg, 2 * kp:2 * kp + 2, :], rhs,
                                    start=(kp == 0), stop=(kp == NG // 2 - 1),
                                    perf_mode=DR,
                                )
                        else:
                            for c in range(ND):
                                nc.tensor.matmul(
                                    pg[:], wgbT[:, gg, c, :], iqT[blk][:, c, :],
                                    start=(c == 0), stop=False,
                                    skip_group_check=True,
                                )
                            for kp in range(ND // 2):
                                nc.tensor.matmul(
                                    pg[:], wgT[:, gg, ND + 2 * kp:ND + 2 * kp + 2, :],
                                    hqT[blk][:, 2 * kp:2 * kp + 2, :],
                                    start=False, stop=(kp == ND // 2 - 1),
                                    perf_mode=DR, skip_group_check=True,
                                )
                        gate = gpool.tile([P, 4 * P], f32r,
                                          tag=("ig" if gg < ND else "fg"))
                        nc.scalar.activation(gate[:], pg[:], Sigm,
                                             bias=bgT[:, gg:gg + 1],
                                             scale=1.0 / (SCA * SCW))
                        if gg < ND:
                            ig_sb[blk] = gate
                        else:
                            ot = opool.tile([P, 4 * P], f32, tag="ot")
                            tmp = opool.tile([P, 4 * P], f32r, tag="tmp")
                            nc.vector.tensor_mul(
                                tmp[:], ig_sb[blk][:], iqT[blk][:, gp, :])
                            nc.vector.tensor_mul(
                                ot[:], gate[:], hfT[blk][:, gp, :])
                            nc.vector.tensor_add(ot[:], ot[:], tmp[:])
                            nc.scalar.dma_start(o[blk, gp], ot[:])

            scan_block(0)
            scan_block(1)
            for blk in range(NBLK):
                ffn_block(blk)
                gate_block(blk)
                if blk + 2 < NBLK:
                    scan_block(blk + 2)

    nc.finalize()
    return nc


_CACHED = {}
_last_result = None


def _host_prep(iQ, iV, W1, b1, W2, b2, Wg, bg, gate_fp8):
    import ml_dtypes
    e4 = ml_dtypes.float8_e4m3
    bf = ml_dtypes.bfloat16

    def q8(x, scale):
        return np.clip(np.asarray(x, np.float32) * scale,
                       -240.0, 240.0).astype(e4)

    def slabs(W, n):
        return np.ascontiguousarray(
            W.reshape(n, P, n, P).transpose(2, 1, 0, 3))

    consts, oband = _host_constants()
    w1q = q8(slabs(W1, ND), SCW)
    w2q = q8(slabs(W2, ND), SCW)
    wgq = q8(slabs(Wg, NG), SCW)
    common = {
        "w1s": w1q, "w2s": w2q, "wgs": wgq,
        "b1c": np.ascontiguousarray((SCA * b1).reshape(ND, P).T),
        "b2cr": np.ascontiguousarray(b2.reshape(ND, P).T),
        "b2cs": np.ascontiguousarray((SCA * b2).reshape(ND, P).T),
        "bgc": np.ascontiguousarray(bg.reshape(NG, P).T),
        "oband": oband,
    }
    if not gate_fp8:
        import ml_dtypes as md
        wgb = np.ascontiguousarray(
            (1024.0 * Wg[:D]).reshape(ND, P, NG, P).transpose(2, 1, 0, 3)
        ).astype(md.bfloat16)
        common["wgb"] = wgb

    zpre = np.zeros((NCH, P, D), bf)
    vb = iV.astype(bf)
    in_maps = []
    for core in range(8):
        b, half = core // 2, core % 2
        ltri_h, mask_h = consts[half]
        Qh = iQ[b, half * T:(half + 1) * T]          # [T, D]
        qt = np.ascontiguousarray(
            Qh.reshape(NBLK, 4 * P, ND, P).transpose(0, 2, 3, 1))
        in_maps.append({
            "v": np.ascontiguousarray(
                vb[b, half * T:(half + 1) * T].reshape(NCH, P, D)),
            "vpre": (np.ascontiguousarray(vb[b, :T].reshape(NCH, P, D))
                     if half == 1 else zpre),
            "iqt": qt,
            "iqt8": q8(qt, SCA),
            "ltri": ltri_h, "maskd": mask_h,
            **common,
        })
    return in_maps


def kernel(iQ, iV, W1, b1, W2, b2, Wg, bg):
    import os
    import sys
    if '/opt/trn_rl_repo' not in sys.path:
        sys.path.insert(0, '/opt/trn_rl_repo')
    from concourse.bass_utils import run_bass_kernel_spmd

    gate_fp8 = os.environ.get('BASS_GATE_MODE', 'fp8' if GATE_FP8 else 'mixed') == 'fp8'

    iQ = np.asarray(iQ, np.float32)
    iV = np.asarray(iV, np.float32)
    W1 = np.asarray(W1, np.float32)
    b1 = np.asarray(b1, np.float32)
    W2 = np.asarray(W2, np.float32)
    b2 = np.asarray(b2, np.float32)
    Wg = np.asarray(Wg, np.float32)
    bg = np.asarray(bg, np.float32)

    key = ('fp8' if gate_fp8 else 'mixed')
    if key not in _CACHED:
        _CACHED[key] = _build_program(gate_fp8)
    nc = _CACHED[key]

    in_maps = _host_prep(iQ, iV, W1, b1, W2, b2, Wg, bg, gate_fp8)

    res = run_bass_kernel_spmd(nc, in_maps, core_ids=list(range(8)))
    global _last_result
    _last_result = res

    out = np.empty((B, S, D), np.float32)
    for core in range(8):
        b, half = core // 2, core % 2
        oT = res.results[core]["o"]                  # [NBLK, ND, P, 4P]
        out[b, half * T:(half + 1) * T] = (
            oT.transpose(0, 3, 1, 2).reshape(T, D))
    return out
